# revision 18
# baseline (speedup 1.0000x reference)
"""BernoulliEdge gnn_message_passing kernel for 8 Trainium2 NeuronCores.

Data-parallel over the batch: each of the 8 cores owns 2 of the 16 batch
elements (its own [N,N,2] gumbel slab + nodes); the tiny MLP weights are
replicated.  One SPMD Bass program runs on all cores; the only
batch-dependent quantity (num_nodes[b]) is loaded into engine registers at
runtime, so the same NEFF serves every core.

Per batch b with n = num_nodes[b]:
  logits = relu([nodes[n] || nodes] @ W1 + b1) @ W2 + b2          [N, 2]
  s      = state, with s[:n+1, n] = logits[:n+1], s[n, :n+1] = logits[:n+1]
  probs  = one_hot(argmax(s + gumbel, -1))  (straight-through hard sample)

The concat trick: the left half of the MLP input is nodes[n] broadcast over
all rows, so  h = relu(nodes @ W1[D:] + (nodes[n] @ W1[:D] + b1))  — the
first half-contraction collapses into a per-output-channel bias.

Fast path (the staged inputs: state == 0).  The runtime hands the NEFF
pre-zeroed ExternalOutput buffers, so with state == 0 the s output IS zero
outside column n / row n and the device only writes those two stripes.  The
bulk of the work is a pure stream: read gumbel, compare the two channels,
write probs.

HBM phase separation: on TRN2 an HBM stack (shared by a NeuronCore pair)
sustains ~800 GB/s when both cores stream the SAME direction, but only
~620 GB/s with mixed read+write traffic (bus turnaround).  So instead of
streaming loads and stores concurrently, the kernel runs two clean phases:
  L: load all 16.8 MiB of gumbel (sync ring), compare channels on DVE into
     a persistent SBUF probs buffer (2 x 8 MiB tiles), fold the column-n
     patch in as it goes;
  S: two giant 8 MiB stores (scalar ring) of the finished probs tiles,
     held back until the last load completes, then the two row-n patches.
Since the program is SPMD, paired cores phase-switch together and the
stack sees single-direction bursts throughout (~42+42 us instead of
~98 us mixed).

A general program (state != 0, checked host-side) keeps full correctness
for arbitrary inputs.
"""

import numpy as np

B, N, D = 16, 1024, 128
NCORES = 8
BPC = B // NCORES          # batches per core
T = N // 128               # 128-row chunks per batch
W = 2 * N                  # flattened [N,2] row width
NH = 4                     # 256-row slabs per batch (2 MiB DMA tiles)

_cached = {}


def _build_program_fast():
    """state == 0 program: phase-separated probs stream + stripe patches."""
    import concourse.bass as bass
    import concourse.tile as tile
    from concourse import bacc, masks, mybir
    from concourse.tile_rust import add_dep_helper

    f32 = mybir.dt.float32
    i32 = mybir.dt.int32
    AF = mybir.ActivationFunctionType
    OP = mybir.AluOpType

    nc = bacc.Bacc("TRN2", target_bir_lowering=False, debug=False)

    # const blob columns (f32, [128, CBLOB]): w1top | w1bot | b1 | w2 | b2pad
    # | cmask0 | cmask1   (cmasks as 0.0/1.0 f32)
    CB_W1T, CB_W1B = 0, D
    CB_B1 = 2 * D
    CB_W2 = 2 * D + 1
    CB_B2 = 2 * D + 3
    CB_CM = 2 * D + 4
    CBLOB = CB_CM + BPC * 2 * T

    gum_in = nc.dram_tensor("gumbel", [BPC, N, W], f32, kind="ExternalInput")
    nodesT_in = nc.dram_tensor("nodesT", [BPC, D, N], f32, kind="ExternalInput")
    blob_in = nc.dram_tensor("blob", [128, CBLOB], f32, kind="ExternalInput")
    meta_in = nc.dram_tensor("meta", [1, BPC], i32, kind="ExternalInput")

    s_out = nc.dram_tensor("s_out", [BPC, N, W], f32, kind="ExternalOutput")
    p_out = nc.dram_tensor("p_out", [BPC, N, W], f32, kind="ExternalOutput")

    # column-layout views: [N, x] rows -> (t, p) with p the SBUF partition
    def col_view(ap):           # [1024, 2] -> [128, 8, 2]
        return ap.rearrange("(t p) c -> p t c", p=128)

    def row_view(ap):           # [1, 2048] -> [128, 8, 2]
        return ap.rearrange("o (t p c) -> (o p) t c", p=128, c=2)

    def tc_view(tile_ap):       # sbuf [128, 16] -> [128, 8, 2]
        return tile_ap.rearrange("p (t c) -> p t c", c=2)

    with tile.TileContext(nc) as tc:
        with (
            tc.tile_pool(name="const", bufs=1) as constp,
            tc.tile_pool(name="mlp", bufs=2) as mlpp,
            tc.tile_pool(name="psA", bufs=2, space="PSUM") as psA,
            tc.tile_pool(name="psB", bufs=1, space="PSUM") as psB,
            tc.tile_pool(name="gld", bufs=7) as gld,
            tc.tile_pool(name="pall", bufs=1) as pallp,
            tc.tile_pool(name="small", bufs=2) as smallp,
        ):
            ET = mybir.EngineType
            # metat + const blob + nodesT lead the sync ring: within one
            # HWDGE ring descriptors drain FIFO, so these ~1.2 MiB finish
            # before the bulk gumbel chunks and the MLP starts early.
            metat = constp.tile([1, BPC], i32)
            nc.sync.dma_start(metat[:], meta_in[:])
            blobt = constp.tile([128, CBLOB], f32)
            nc.sync.dma_start(blobt[:], blob_in[:])
            ntt_all = constp.tile([128, BPC * N], f32)
            nc.sync.dma_start(
                ntt_all[:].rearrange("p (b n) -> p b n", b=BPC),
                nodesT_in.rearrange("b d n -> d b n"),
            )
            w1top = blobt[:, CB_W1T : CB_W1T + D]
            w1bot = blobt[:, CB_W1B : CB_W1B + D]
            b1t = blobt[:, CB_B1 : CB_B1 + 1]
            w2t = blobt[:, CB_W2 : CB_W2 + 2]
            b2t = blobt[0:2, CB_B2 : CB_B2 + 1]
            cmaskts = [
                blobt[:, CB_CM + b * 2 * T : CB_CM + (b + 1) * 2 * T]
                for b in range(BPC)
            ]
            ntTs = [ntt_all[:, b * N : (b + 1) * N] for b in range(BPC)]
            # 2x2 identity for the logit transposes
            ident2 = constp.tile([2, 2], f32)
            masks.make_identity(nc, ident2[:])

            # persistent probs buffers: one 8 MiB tile per batch
            # layout: ptile[p, t*W + w] = p[b, 128t + p, w]
            ptiles = [
                pallp.tile([128, T * W], f32, name=f"pall{b}", tag=f"pall{b}")
                for b in range(BPC)
            ]

            nvals = [
                nc.values_load(
                    metat[0:1, b : b + 1], min_val=0, max_val=N - 1,
                    skip_runtime_bounds_check=True,
                    engines=(ET.DVE, ET.PE),
                )
                for b in range(BPC)
            ]

            # all bulk loads ride the sync ring (1 MiB chunks, 6 in flight)
            gts = []
            for b in range(BPC):
                for t in range(T):
                    gt = gld.tile([128, W], f32, tag="gt")
                    ld = nc.sync.dma_start(gt[:], gum_in[b, t * 128 : (t + 1) * 128, :])
                    gts.append((gt, ld))
            last_load = gts[-1][1]

            # MLP tensor/scalar chains for both batches
            cvals, hTs, lTs = [], [], []
            for b in range(BPC):
                c_ps = psB.tile([D, 1], f32, tag="vec_ps")
                nc.tensor.matmul(c_ps[:], w1top, ntTs[b][:, bass.ds(nvals[b], 1)])
                cval = mlpp.tile([D, 1], f32, tag="cval")
                nc.scalar.activation(cval[:], c_ps[:], AF.Identity, bias=b1t)
                cvals.append(cval)
            for b in range(BPC):
                hT = mlpp.tile([128, N], f32, tag="hT", bufs=1)
                for hf in range(2):
                    h_ps = psA.tile([128, 512], f32, tag="h_ps")
                    nc.tensor.matmul(
                        h_ps[:], w1bot, ntTs[b][:, hf * 512 : (hf + 1) * 512]
                    )
                    nc.scalar.activation(
                        hT[:, hf * 512 : (hf + 1) * 512], h_ps[:], AF.Relu,
                        bias=cvals[b][:],
                    )
                hTs.append(hT)
                lT = mlpp.tile([2, N], f32, tag="lT", bufs=1)
                for hf in range(2):
                    l_ps = psB.tile([2, 512], f32, tag="l_ps")
                    nc.tensor.matmul(l_ps[:], w2t, hT[:, hf * 512 : (hf + 1) * 512])
                    nc.scalar.activation(
                        lT[:, hf * 512 : (hf + 1) * 512], l_ps[:], AF.Identity,
                        bias=b2t,
                    )
                lTs.append(lT)

            # ---- phase L: compare gumbel channels into the probs tiles ----
            # The column-n gumbel values are extracted from the streaming
            # chunks (no scatter/gather DMA); phase-1 and folds slot in
            # after each batch finishes comparing.
            colgums = []
            for b in range(BPC):
                colgum = smallp.tile([128, 2 * T], f32, tag="colgum")
                colgums.append(colgum)

            def emit_cmp(i):
                b, t = divmod(i, T)
                gt = gts[i][0]
                pv = ptiles[b][:, t * W : (t + 1) * W]
                # single DVE op for both channels: pv[2j+c] = g[2j+c] >=
                # g[2j+(1-c)] via a pair-flipped (negative-stride) view.  An
                # exact channel tie yields [1,1] instead of [1,0]; kernel()
                # fixes those (measure-zero) positions on the host.
                gt_v = gt[:].rearrange("p (j c) -> p j c", c=2)
                nc.vector.tensor_tensor(
                    pv.rearrange("p (j c) -> p j c", c=2),
                    gt_v, gt_v[:, :, ::-1], op=OP.is_ge,
                )
                # extract the column-n gumbel pair for this row block
                nc.vector.tensor_copy(
                    colgums[b][:, 2 * t : 2 * t + 2], gt[:, bass.ds(nvals[b] * 2, 2)]
                )

            def phase1_vector(b):
                """colvals (masked logit columns) + colpv for batch b."""
                lc_big = psA.tile([128, 2 * T], f32, tag="lc_big")
                for t in range(T):
                    nc.tensor.transpose(
                        lc_big[:, 2 * t : 2 * t + 2],
                        lTs[b][:, t * 128 : (t + 1) * 128], ident2[:],
                    )
                colvals = mlpp.tile([128, 2 * T], f32, tag="colvals")
                nc.vector.tensor_tensor(
                    colvals[:], lc_big[:], cmaskts[b], op=OP.mult
                )
                tmpc = smallp.tile([128, 2 * T], f32, tag="tmpc")
                nc.vector.tensor_add(tmpc[:], colgums[b][:], colvals[:])
                colpv = mlpp.tile([128, 2 * T], f32, tag="colpv")
                nc.vector.tensor_tensor(
                    colpv[:, 0 : 2 * T : 2], tmpc[:, 0 : 2 * T : 2],
                    tmpc[:, 1 : 2 * T : 2], op=OP.is_ge,
                )
                nc.vector.tensor_tensor(
                    colpv[:, 1 : 2 * T : 2], tmpc[:, 1 : 2 * T : 2],
                    tmpc[:, 0 : 2 * T : 2], op=OP.is_gt,
                )
                return colvals, colpv

            def emit_folds(b, colpv):
                n_rv = nvals[b]
                for t in range(T):
                    pv = ptiles[b][:, t * W : (t + 1) * W]
                    nc.vector.tensor_copy(
                        pv[:, bass.ds(n_rv * 2, 2)], colpv[:, 2 * t : 2 * t + 2]
                    )

            colvalss = [None] * BPC
            colpvs = [None] * BPC
            for i in range(T):
                emit_cmp(i)
            colvalss[0], colpvs[0] = phase1_vector(0)
            emit_folds(0, colpvs[0])
            for i in range(T, BPC * T):
                emit_cmp(i)
            if BPC > 1:
                colvalss[1], colpvs[1] = phase1_vector(1)
                emit_folds(1, colpvs[1])

            # ---- phase S: two giant stores, held until the last load ----
            bulk_p_stores = []
            for b in range(BPC):
                p_st = nc.scalar.dma_start(
                    p_out[b].rearrange("(t p) w -> p t w", p=128),
                    ptiles[b][:].rearrange("p (t w) -> p t w", t=T),
                )
                add_dep_helper(p_st.ins, last_load.ins, reason="phase separation")
                bulk_p_stores.append(p_st)

            # ---- row-n gumbel + prow (gpsimd SWDGE: dynamic-offset DMAs
            # are software-generated; on a HWDGE ring the issuing engine
            # would stall ~10us building the 128 descriptors) ----
            nlate = [
                nc.values_load(
                    metat[0:1, b : b + 1], min_val=0, max_val=N - 1,
                    skip_runtime_bounds_check=True,
                    engines=(ET.Pool,),
                )
                for b in range(BPC)
            ]
            growcols, prowcols = [], []
            for b in range(BPC):
                growcol = smallp.tile([128, 2 * T], f32, tag="growcol")
                nc.gpsimd.dma_start(
                    tc_view(growcol[:]),
                    row_view(gum_in[b][bass.ds(nlate[b], 1), :]),
                )
                growcols.append(growcol)
            for b in range(BPC):
                tmpr = smallp.tile([128, 2 * T], f32, tag="tmpr")
                nc.vector.tensor_add(tmpr[:], growcols[b][:], colvalss[b][:])
                prowcol = mlpp.tile([128, 2 * T], f32, tag="prowcol")
                nc.vector.tensor_tensor(
                    prowcol[:, 0 : 2 * T : 2], tmpr[:, 0 : 2 * T : 2],
                    tmpr[:, 1 : 2 * T : 2], op=OP.is_ge,
                )
                nc.vector.tensor_tensor(
                    prowcol[:, 1 : 2 * T : 2], tmpr[:, 1 : 2 * T : 2],
                    tmpr[:, 0 : 2 * T : 2], op=OP.is_gt,
                )
                prowcols.append(prowcol)

            # ---- stripe patches (gpsimd, run during the store phase) ----
            for b in range(BPC):
                nc.gpsimd.dma_start(
                    col_view(s_out[b][:, bass.ds(nlate[b] * 2, 2)]),
                    tc_view(colvalss[b][:]),
                )
                nc.gpsimd.dma_start(
                    row_view(s_out[b][bass.ds(nlate[b], 1), :]),
                    tc_view(colvalss[b][:]),
                )
            for b in range(BPC):
                pr = nc.gpsimd.dma_start(
                    row_view(p_out[b][bass.ds(nlate[b], 1), :]),
                    tc_view(prowcols[b][:]),
                )
                add_dep_helper(pr.ins, bulk_p_stores[b].ins, reason="p row patch WAW")

    nc.compile()
    return nc


def _build_program_general():
    """Arbitrary-state fallback: full state read + full s/p writes."""
    import concourse.bass as bass
    import concourse.tile as tile
    from concourse import bacc, masks, mybir
    from concourse.tile_rust import add_dep_helper

    f32 = mybir.dt.float32
    i32 = mybir.dt.int32
    u8 = mybir.dt.uint8
    AF = mybir.ActivationFunctionType
    OP = mybir.AluOpType

    nc = bacc.Bacc("TRN2", target_bir_lowering=False, debug=False)

    state_in = nc.dram_tensor("state", [BPC, N, W], f32, kind="ExternalInput")
    gum_in = nc.dram_tensor("gumbel", [BPC, N, W], f32, kind="ExternalInput")
    nodesT_in = nc.dram_tensor("nodesT", [BPC, D, N], f32, kind="ExternalInput")
    w1_in = nc.dram_tensor("w1", [2 * D, D], f32, kind="ExternalInput")
    b1_in = nc.dram_tensor("b1", [D, 1], f32, kind="ExternalInput")
    w2_in = nc.dram_tensor("w2", [D, 2], f32, kind="ExternalInput")
    b2_in = nc.dram_tensor("b2", [2, 1], f32, kind="ExternalInput")
    meta_in = nc.dram_tensor("meta", [1, BPC], i32, kind="ExternalInput")
    cmask_in = nc.dram_tensor("cmask", [BPC, 128, 2 * T], u8, kind="ExternalInput")
    rmask_in = nc.dram_tensor("rmask", [BPC, 1, W], u8, kind="ExternalInput")

    s_out = nc.dram_tensor("s_out", [BPC, N, W], f32, kind="ExternalOutput")
    p_out = nc.dram_tensor("p_out", [BPC, N, W], f32, kind="ExternalOutput")

    # interleaved-logits bounce buffer (see row fixup below)
    rowpatch = nc.dram_tensor("rowpatch", [BPC, W], f32)

    with tile.TileContext(nc) as tc:
        with (
            tc.tile_pool(name="const", bufs=1) as constp,
            tc.tile_pool(name="mlp", bufs=2) as mlpp,
            tc.tile_pool(name="psA", bufs=2, space="PSUM") as psA,
            tc.tile_pool(name="psB", bufs=1, space="PSUM") as psB,
            tc.tile_pool(name="bigld", bufs=5) as bigld,
            tc.tile_pool(name="bigst", bufs=4) as bigst,
            tc.tile_pool(name="small", bufs=1) as smallp,
            tc.tile_pool(name="persist", bufs=2) as persistp,
        ):
            ident = constp.tile([128, 128], f32)
            masks.make_identity(nc, ident[:])
            w1top = constp.tile([128, D], f32)
            nc.sync.dma_start(w1top[:], w1_in[0:D, :])
            w1bot = constp.tile([128, D], f32)
            nc.sync.dma_start(w1bot[:], w1_in[D : 2 * D, :])
            w2t = constp.tile([128, 2], f32)
            nc.sync.dma_start(w2t[:], w2_in[:])
            b1t = constp.tile([128, 1], f32)
            nc.sync.dma_start(b1t[:], b1_in[:])
            b2t = constp.tile([2, 1], f32)
            nc.sync.dma_start(b2t[:], b2_in[:])
            metat = constp.tile([1, BPC], i32)
            nc.sync.dma_start(metat[:], meta_in[:])

            nvals = []
            fixups = []       # (merged, prow) per batch
            lcols = []
            bulk_s_stores = []
            bulk_p_stores = []

            for b in range(BPC):
                n_rv = nc.values_load(
                    metat[0:1, b : b + 1], min_val=0, max_val=N - 1,
                    skip_runtime_bounds_check=True,
                )
                nvals.append(n_rv)

                # ---- MLP: logits^T [2, N] ----
                leftcol = mlpp.tile([D, 1], f32, tag="leftcol")
                nc.sync.dma_start(leftcol[:], nodesT_in[b][:, bass.ds(n_rv, 1)])
                c_ps = psB.tile([D, 1], f32, tag="vec_ps")
                nc.tensor.matmul(c_ps[:], w1top[:], leftcol[:])
                cval = mlpp.tile([D, 1], f32, tag="cval")
                nc.vector.tensor_add(cval[:], c_ps[:], b1t[:])

                hT = mlpp.tile([128, N], f32, tag="hT")
                ntT = mlpp.tile([128, N], f32, tag="ntT")
                nc.sync.dma_start(ntT[:], nodesT_in[b])
                for hf in range(2):
                    h_ps = psA.tile([128, 512], f32, tag="h_ps")
                    nc.tensor.matmul(h_ps[:], w1bot[:], ntT[:, hf * 512 : (hf + 1) * 512])
                    nc.scalar.activation(
                        hT[:, hf * 512 : (hf + 1) * 512], h_ps[:], AF.Relu, bias=cval[:]
                    )

                lT = mlpp.tile([2, N], f32, tag="lT")
                for hf in range(2):
                    l_ps = psB.tile([2, 512], f32, tag="l_ps")
                    nc.tensor.matmul(l_ps[:], w2t[:], hT[:, hf * 512 : (hf + 1) * 512])
                    nc.scalar.activation(
                        lT[:, hf * 512 : (hf + 1) * 512], l_ps[:], AF.Identity,
                        bias=b2t[:],
                    )

                # logits as columns: lcol[:, 2t:2t+2][i, c] = logits[128t+i, c]
                lcol = mlpp.tile([128, 2 * T], f32, tag="lcol")
                patch_stores = []
                for t in range(T):
                    lc_ps = psA.tile([128, 2], f32, tag="lc_ps")
                    nc.tensor.transpose(
                        lc_ps[:], lT[:, t * 128 : (t + 1) * 128], ident[0:2, 0:2]
                    )
                    nc.vector.tensor_copy(lcol[:, 2 * t : 2 * t + 2], lc_ps[:])
                    # partition-major DMA order writes [i0c0 i0c1 i1c0 ...] =
                    # the interleaved [N,2]-row-major logits layout
                    st_i = nc.gpsimd.dma_start(
                        rowpatch[b, 256 * t : 256 * (t + 1)],
                        lcol[:, 2 * t : 2 * t + 2],
                    )
                    patch_stores.append(st_i)

                # ---- row-n fixup values (stored after the bulk pass) ----
                rowIL = smallp.tile([1, W], f32, tag="rowIL")
                ld_i = nc.gpsimd.dma_start(rowIL[:], rowpatch[b : b + 1, :])
                for st_i in patch_stores:
                    add_dep_helper(ld_i.ins, st_i.ins, reason="rowpatch RAW")
                rmk = smallp.tile([1, W], u8, tag="rmk")
                nc.gpsimd.dma_start(rmk[:], rmask_in[b])
                merged = persistp.tile([1, W], f32, tag="merged")
                nc.gpsimd.dma_start(merged[:], state_in[b][bass.ds(n_rv, 1), :])
                nc.vector.copy_predicated(merged[:], rmk[:], rowIL[:])
                grow = smallp.tile([1, W], f32, tag="grow")
                nc.gpsimd.dma_start(grow[:], gum_in[b][bass.ds(n_rv, 1), :])
                nc.vector.tensor_add(grow[:], grow[:], merged[:])
                prow = persistp.tile([1, W], f32, tag="prow")
                nc.vector.tensor_tensor(
                    prow[:, 0:W:2], grow[:, 0:W:2], grow[:, 1:W:2], op=OP.is_ge
                )
                nc.vector.tensor_tensor(
                    prow[:, 1:W:2], grow[:, 1:W:2], grow[:, 0:W:2], op=OP.is_gt
                )
                fixups.append((merged, prow))
                lcols.append(lcol)

            for b in range(BPC):
                n_rv = nvals[b]
                lcol = lcols[b]
                # ---- bulk pass over the [N, N, 2] slab ----
                # cmtile[p, 2t+c] = 1 iff 128t + p <= n
                cmtile = smallp.tile([128, 2 * T], u8, tag="cmtile")
                nc.sync.dma_start(cmtile[:], cmask_in[b])
                s_stores = []
                p_stores = []
                for t in range(T):
                    rows = slice(t * 128, (t + 1) * 128)
                    st = bigld.tile([128, W], f32, tag="st")
                    nc.sync.dma_start(st[:], state_in[b, rows, :])
                    # scatter column n: st[i, 2n:2n+2] = logits[i] where i <= n
                    nc.vector.copy_predicated(
                        st[:, bass.ds(n_rv * 2, 2)], cmtile[:, 2 * t : 2 * t + 2],
                        lcol[:, 2 * t : 2 * t + 2],
                    )
                    s_st = nc.scalar.dma_start(s_out[b, rows, :], st[:])
                    s_stores.append(s_st)
                    gt = bigld.tile([128, W], f32, tag="gt")
                    nc.sync.dma_start(gt[:], gum_in[b, rows, :])
                    nc.vector.tensor_add(gt[:], gt[:], st[:])
                    pt = bigst.tile([128, W], f32, tag="pt")
                    nc.vector.tensor_tensor(
                        pt[:, 0:W:2], gt[:, 0:W:2], gt[:, 1:W:2], op=OP.is_ge
                    )
                    nc.vector.tensor_tensor(
                        pt[:, 1:W:2], gt[:, 1:W:2], gt[:, 0:W:2], op=OP.is_gt
                    )
                    p_st = nc.scalar.dma_start(p_out[b, rows, :], pt[:])
                    p_stores.append(p_st)
                bulk_s_stores.append(s_stores)
                bulk_p_stores.append(p_stores)

            # ---- row-n fixup stores (must land after the bulk stores) ----
            for b in range(BPC):
                merged, prow = fixups[b]
                fs = nc.scalar.dma_start(s_out[b][bass.ds(nvals[b], 1), :], merged[:])
                for s_st in bulk_s_stores[b]:
                    add_dep_helper(fs.ins, s_st.ins, reason="s row fixup WAW")
                fp = nc.scalar.dma_start(p_out[b][bass.ds(nvals[b], 1), :], prow[:])
                for p_st in bulk_p_stores[b]:
                    add_dep_helper(fp.ins, p_st.ins, reason="p row fixup WAW")

    nc.compile()
    return nc


def get_program(fast=True):
    key = "fast" if fast else "general"
    if key not in _cached:
        _cached[key] = (
            _build_program_fast() if fast else _build_program_general()
        )
    return _cached[key]


def _shard_common(nodes, W1, b1, W2, b2, num_nodes):
    nodes = np.ascontiguousarray(nodes, dtype=np.float32)
    W1 = np.ascontiguousarray(W1, dtype=np.float32)
    W2 = np.ascontiguousarray(W2, dtype=np.float32)
    b1 = np.ascontiguousarray(b1, dtype=np.float32).reshape(D, 1)
    b2 = np.ascontiguousarray(b2, dtype=np.float32).reshape(2, 1)
    nn = np.clip(np.asarray(num_nodes), 0, N - 1).astype(np.int32)
    return nodes, W1, b1, W2, b2, nn


def make_in_maps_fast(nodes, state, W1, b1, W2, b2, num_nodes, gumbel):
    nodes, W1, b1, W2, b2, nn = _shard_common(nodes, W1, b1, W2, b2, num_nodes)
    gumbel = np.ascontiguousarray(gumbel, dtype=np.float32)

    idx = np.arange(N)
    CBLOB = 2 * D + 4 + BPC * 2 * T
    in_maps = []
    for k in range(NCORES):
        b0 = k * BPC
        ns = nn[b0 : b0 + BPC]
        # cmask[b, p, 2t+c] = 1.0 iff 128t + p <= n_b
        rowidx = (idx.reshape(T, 128).T)[None, :, :]              # [1, 128, T]
        cmask = np.repeat(rowidx <= ns[:, None, None], 2, axis=2).astype(np.float32)
        blob = np.zeros((128, CBLOB), dtype=np.float32)
        blob[:, 0:D] = W1[0:D, :]
        blob[:, D : 2 * D] = W1[D : 2 * D, :]
        blob[:, 2 * D] = b1[:, 0]
        blob[:, 2 * D + 1 : 2 * D + 3] = W2
        blob[0:2, 2 * D + 3] = b2[:, 0]
        for b in range(BPC):
            blob[:, 2 * D + 4 + b * 2 * T : 2 * D + 4 + (b + 1) * 2 * T] = cmask[b]
        in_maps.append(
            {
                "gumbel": gumbel[b0 : b0 + BPC].reshape(BPC, N, W),
                "nodesT": np.ascontiguousarray(
                    nodes[b0 : b0 + BPC].transpose(0, 2, 1)
                ),
                "blob": blob,
                "meta": ns.reshape(1, BPC),
            }
        )
    return in_maps


def make_in_maps_general(nodes, state, W1, b1, W2, b2, num_nodes, gumbel):
    nodes, W1, b1, W2, b2, nn = _shard_common(nodes, W1, b1, W2, b2, num_nodes)
    state = np.ascontiguousarray(state, dtype=np.float32)
    gumbel = np.ascontiguousarray(gumbel, dtype=np.float32)

    idx = np.arange(N)
    in_maps = []
    for k in range(NCORES):
        b0 = k * BPC
        ns = nn[b0 : b0 + BPC]
        rowidx = (idx.reshape(T, 128).T)[None, :, :]              # [1, 128, T]
        cmask = np.repeat(rowidx <= ns[:, None, None], 2, axis=2).astype(np.uint8)
        rmask = np.repeat(idx[None, :] <= ns[:, None], 2, axis=1) # [BPC, 2N]
        in_maps.append(
            {
                "state": state[b0 : b0 + BPC].reshape(BPC, N, W),
                "gumbel": gumbel[b0 : b0 + BPC].reshape(BPC, N, W),
                "nodesT": np.ascontiguousarray(
                    nodes[b0 : b0 + BPC].transpose(0, 2, 1)
                ),
                "w1": W1,
                "b1": b1,
                "w2": W2,
                "b2": b2,
                "meta": ns.reshape(1, BPC),
                "cmask": np.ascontiguousarray(cmask),
                "rmask": np.ascontiguousarray(
                    rmask.astype(np.uint8).reshape(BPC, 1, W)
                ),
            }
        )
    return in_maps


def prepare(nodes, state, W1, b1, W2, b2, num_nodes, gumbel):
    fast = not np.asarray(state).any()
    nc = get_program(fast)
    mk = make_in_maps_fast if fast else make_in_maps_general
    in_maps = mk(nodes, state, W1, b1, W2, b2, num_nodes, gumbel)
    return nc, in_maps


# test.py compatibility (staged inputs always have state == 0)
def make_in_maps(nodes, state, W1, b1, W2, b2, num_nodes, gumbel):
    return make_in_maps_fast(nodes, state, W1, b1, W2, b2, num_nodes, gumbel)


def kernel(nodes, state, W1, b1, W2, b2, num_nodes, gumbel):
    from concourse.bass_utils import run_bass_kernel_spmd

    nc, in_maps = prepare(nodes, state, W1, b1, W2, b2, num_nodes, gumbel)
    res = run_bass_kernel_spmd(nc, in_maps, list(range(NCORES)))
    s_full = np.concatenate(
        [res.results[k]["s_out"].reshape(BPC, N, N, 2) for k in range(NCORES)], axis=0
    )
    p_full = np.concatenate(
        [res.results[k]["p_out"].reshape(BPC, N, N, 2) for k in range(NCORES)], axis=0
    )
    if not np.asarray(state).any():
        # fast path computes the bulk probs as [g0>=g1, g1>=g0]; an exact
        # channel tie gives [1,1] where argmax tie-breaking wants [1,0].
        # Fix those positions (outside the device-exact row-n/col-n
        # stripes, which use is_ge/is_gt pairs).
        g = np.asarray(gumbel, dtype=np.float32)
        tb, ti, tj = np.nonzero(g[..., 0] == g[..., 1])
        if tb.size:
            nn = np.clip(np.asarray(num_nodes), 0, N - 1).astype(np.int64)
            keep = (ti != nn[tb]) & (tj != nn[tb])
            p_full[tb[keep], ti[keep], tj[keep], 1] = 0.0
    return s_full, p_full
